# revision 37
# baseline (speedup 1.0000x reference)
"""Trainium2 Bass kernel for the atomic-descriptor builder (radial Chebyshev +
angular Legendre descriptors, N=256 atoms, minimum-image PBC).

Strategy: shard the central-atom axis i across 8 NeuronCores (32 atoms each).
Per core, pairs live as [128 j-partitions, 2 j-chunks x 32 atoms free].
The O(N^3) triplet sum is reformulated exactly via the monomial expansion of
Legendre polynomials: q_ang[i,n,l] = sum_c A[c,l] * M[i,n,c]^2 with
M[i,n,c] = sum_j g[i,j,n] (u_ij)^c over the 35 tensor-power monomials of
degree <= 4 (multinomial weights folded into A on the host).

Key layout/scheduling choices (sim-validated against the TRN2 cost model):
  * one packed DVE op for ds (si replicated / sj per-partition views) and a
    two-op fused-compare minimum-image wrap in fractional coordinates; the
    box scale L is absorbed into downstream scalars (sqrt bias, cutoff and
    Chebyshev-argument constants), so no per-element scaling op exists
  * ACT Sqrt is the only table-based activation (table load hidden in the
    input-DMA shadow behind a dependency-free dummy op); the cosine cutoff
    is a degree-4 polynomial in z=(r/rc)^2 (cos(pi*sqrt(z)/2) is entire in
    z), evaluated on the otherwise idle Pool(GPSIMD)+ACT lanes
  * everything after the f32 distance head runs in fp16: TensorTensor and
    TensorScalar get the 2x DVE mode and PE matmuls cost 8ns each
  * the Chebyshev ladder uses product-doubling with row-batched ops
    (multi-row APs computing (T3,T4), (T5,T7), (T6,T8) pairs at once); the
    Chebyshev argument x = 2zc - 4*min(r,rc)/rc + 1 reuses the cutoff's zc
  * the 8 tensor-power "trios" collapse into 3 wide TensorTensor ops using
    sliding-window access patterns over cyclically-extended component rows
    (41-row stationary with 6 duplicate rows, zero-weighted in A); the
    extension copies run on ACT
  * one PSUM bank [41,32,9] accumulates all 64 moment matmuls (start/stop
    per atom); 64 extra 1-column matmuls against a ones vector produce q_r
    partition-transposed as [9,32] so no single-partition copy is needed
  * device tail is one ACT Square (squared moments), one DVE copy, and a
    single merged-output DMA [41,160] fp16; the tiny [35x5] Legendre fold
    A is applied on the host during the unshard/gather (float64)
"""
import numpy as np

N_ATOMS = 256
NCORES = 8
NI = N_ATOMS // NCORES        # 32 central atoms per core
NCHUNK = 2                    # j-chunks of 128 partitions
W = NCHUNK * NI               # 64 free columns per (chunk, atom)
NFEAT = 9                     # radial features (K_RADIAL+1)
NA = 4                        # angular radial features
RC = 5.0
NCOMP = 41                    # 35 unique monomials + 6 cyclic-dup rows
GRP = 8

# fused f32 input block columns: si (once, broadcast on-device) | sj | mask
C_SI, C_SJ, C_MASK = 0, 96, 102
NCOL = C_MASK + W

# cos(pi*y/2) = sum_k PC[k] * (y^2)^k  (Taylor in z=y^2; entire function,
# |err| < 2.6e-5 on z in [0,1])
_PC = [1.0]
for _k in range(1, 5):
    _PC.append(_PC[-1] * (-(np.pi / 2) ** 2) / ((2 * _k - 1) * (2 * _k)))
PA0, PA1, PA2, PA3, PA4 = [float(v) for v in _PC]

# Legendre-in-monomial coefficients: q_l = sum_p CLP[l][p] * S_p
CLP = np.array([
    [1.0, 0, 0, 0, 0],
    [0, 1.0, 0, 0, 0],
    [-0.5, 0, 1.5, 0, 0],
    [0, -1.5, 0, 2.5, 0],
    [0.375, 0, -3.75, 0, 4.375],
], dtype=np.float64)

# stationary component rows: (degree, multinomial weight); -1 deg = dup row
_ROWS = [(0, 1)] + [(1, 1)] * 3 + [(-1, 0)] * 2 \
    + [(2, 1)] * 3 + [(-1, 0)] * 2 + [(2, 2)] * 3 + [(-1, 0)] * 2 \
    + [(3, 1)] * 3 + [(3, 3)] * 6 \
    + [(4, 1)] * 3 + [(4, 6)] * 3 + [(4, 4)] * 3 + [(4, 12)] * 3 \
    + [(4, 4)] * 3 + [(3, 6)]
assert len(_ROWS) == NCOMP


_compiled = {}


def _build_program(box):
    import concourse.bass as bass
    import concourse.bacc as bacc
    import concourse.tile as tile
    from concourse import mybir

    f32 = mybir.dt.float32
    f16 = mybir.dt.float16
    op = mybir.AluOpType
    act = mybir.ActivationFunctionType

    boxf = np.asarray(box, np.float32)
    diag_box = float(np.abs(boxf - np.diag(np.diag(boxf))).max()) == 0.0
    eq_diag = diag_box and boxf[0, 0] == boxf[1, 1] == boxf[2, 2]
    L = float(boxf[0, 0])

    SCL = L if eq_diag else 1.0   # dsw/rsq stay fractional for eq-diag
    nc = bacc.Bacc("TRN2", target_bir_lowering=False, debug=False,
                   enable_asserts=False)

    insd = nc.dram_tensor("ins", [128, NCOL], f32, kind="ExternalInput")
    outd = nc.dram_tensor("outt", [NCOMP, NI * NA + NI], f16,
                      kind="ExternalOutput")

    def rowap(t, r0, pattern, inner=64, cols=slice(0, W)):
        """AP over tile t starting at row r0 with extra row-structured dims.
        pattern = list of (row_step, count); innermost dim = [1, inner]."""
        base = t[:, r0, cols] if inner != W or cols != slice(0, W) \
            else t[:, r0, :]
        rs = t[:, 1, :].offset - t[:, 0, :].offset
        dims = [base.ap[0]] + [[st * rs, n] for st, n in pattern] \
            + [list(base.ap[-1])]
        return bass.AP(tensor=base.tensor, offset=base.offset, ap=dims)

    with tile.TileContext(nc) as tc:
        with tc.tile_pool(name="sb", bufs=1) as sb, \
             tc.tile_pool(name="ps", bufs=1, space="PSUM") as ps:

            def t(shape, tag, dt=f32):
                return sb.tile(shape, dt, tag=tag, name=tag)

            ins = t([128, NCOL], "ins")
            in_ap = insd.ap()
            nc.sync.dma_start(out=ins[:, 0:C_MASK], in_=in_ap[:, 0:C_MASK])
            nc.sync.dma_start(out=ins[:, C_MASK:], in_=in_ap[:, C_MASK:])
            mask = ins[:, C_MASK:C_MASK + W]

            dsw = t([128, 3, W], "dsw")
            dr2 = t([128, 3, W], "dr2")
            rsq = t([128, W], "rsq")
            rinv = t([128, W], "rinv")
            rij = t([128, W], "rij")
            b_ = t([128, W], "b_")
            zc21 = t([128, W], "zc21")
            Tla = t([128, 9, W], "Tla", f16)   # x T2..T8 | ones
            prods = t([128, 4, W], "prods", f16)
            Tt = t([128, NCOMP, W], "Tt", f16)
            mov = t([128, NFEAT, W], "mov", f16)
            zc = t([128, W], "zc")
            z2 = t([128, W], "z2")
            e0 = t([128, W], "e0")
            e1 = t([128, W], "e1")
            f1 = t([128, W], "f1")
            p_ = t([128, W], "p_")
            cv = t([128, W], "cv")
            maskc = t([128, W], "maskc")
            hm = t([128, W], "hm")
            h = t([128, W], "h", f16)
            OT = t([NCOMP, NI * NA + NI], "OT", f16)
            ones1 = t([128, 1], "ones1", f16)
            b_eps = t([128, 1], "b_eps")

            pm = ps.tile([NCOMP, NI, NFEAT], mybir.dt.float32, tag="pm",
                         name="pm")
            qrT = ps.tile([NFEAT, NI], mybir.dt.float32, tag="qrT",
                          name="qrT")

            # ---- constants (Pool memsets; run in the input-DMA shadow) ----
            nc.gpsimd.memset(Tla[:, 8, :], 1.0)
            nc.gpsimd.memset(Tt[:, 0, :], 1.0)
            nc.gpsimd.memset(b_eps, 1e-12 / SCL ** 2)
            nc.gpsimd.memset(ones1, 1.0)
            nc.gpsimd.memset(OT, 0.0)
            # dep-free first ACT op: forces the single act-table load to run
            # inside the input-DMA shadow instead of behind the rsq wait
            nc.scalar.activation(out=f1[:, 0:1], in_=b_eps[:, :],
                                 func=act.Sqrt, bias=b_eps[:, :])

            # ---- distances (f32 head, DVE) ---------------------------
            # ds = si'' - sj'' in prescaled coords L*(s+1/2) / L*s;
            # minimum image via one python_mod tensor_scalar.
            si_v = bass.AP(tensor=ins[:, :].tensor,
                           offset=ins[:, C_SI:C_SI + 1].offset,
                           ap=[ins[:, :].ap[0], [NI, 3], [0, 2], [1, NI]])
            sj_v = bass.AP(tensor=ins[:, :].tensor,
                           offset=ins[:, C_SJ:C_SJ + 1].offset,
                           ap=[ins[:, :].ap[0], [2, 3], [1, 2], [0, NI]])
            ds4 = bass.AP(tensor=dsw[:, :, :].tensor,
                          offset=dsw[:, :, :].offset,
                          ap=[dsw[:, :, :].ap[0], [W, 3], [NI, 2], [1, NI]])
            nc.vector.tensor_tensor(out=ds4, in0=si_v, in1=sj_v,
                                    op=op.subtract)
            # minimum-image wrap in fractional coords: two fused compare ops
            # (ds>=.5)-ds then (ds<=-.5)-X  == ds - round(ds)
            wrX = dr2                       # reuse dr2 as scratch
            nc.vector.scalar_tensor_tensor(
                out=wrX[:, :, :], in0=dsw[:, :, :], scalar=0.5,
                in1=dsw[:, :, :], op0=op.is_ge, op1=op.subtract)
            nc.vector.scalar_tensor_tensor(
                out=dsw[:, :, :], in0=dsw[:, :, :], scalar=-0.5,
                in1=wrX[:, :, :], op0=op.is_le, op1=op.subtract)
            if not diag_box:
                # general box: dr = B @ ds (fractional wrap already done)
                drt = t([128, 3, W], "drt")
                for d in range(3):
                    nc.vector.tensor_scalar(
                        out=drt[:, d, :], in0=dsw[:, 0, :],
                        scalar1=float(boxf[d, 0]), scalar2=None, op0=op.mult)
                    for e in (1, 2):
                        nc.vector.scalar_tensor_tensor(
                            out=drt[:, d, :], in0=dsw[:, e, :],
                            scalar=float(boxf[d, e]), in1=drt[:, d, :],
                            op0=op.mult, op1=op.add)
                dsw = drt
            elif not eq_diag:
                for d in range(3):
                    nc.vector.tensor_scalar(
                        out=dsw[:, d, :], in0=dsw[:, d, :],
                        scalar1=float(boxf[d, d]), scalar2=None, op0=op.mult)
            nc.vector.tensor_tensor(out=dr2[:, :, :], in0=dsw[:, :, :],
                                    in1=dsw[:, :, :], op=op.mult)
            nc.vector.tensor_reduce(
                out=rsq[:, :], in_=dr2[:, :, :].rearrange("p d w -> p w d"),
                axis=mybir.AxisListType.X, op=op.add)

            # ---- juncture: sqrt (ACT) + reciprocal + unit vectors ------
            nc.scalar.activation(out=rij[:, :], in_=rsq[:, :],
                                 func=act.Sqrt, bias=b_eps[:, :])
            nc.vector.reciprocal(out=rinv[:, :], in_=rij[:, :])
            rinv_b = bass.AP(tensor=rinv[:, :].tensor,
                             offset=rinv[:, :].offset,
                             ap=[rinv[:, :].ap[0], [0, 3], [1, W]])
            nc.vector.tensor_tensor(out=Tt[:, 1:4, :], in0=dsw[:, :, :],
                                    in1=rinv_b, op=op.mult)        # u
            nc.scalar.activation(out=Tt[:, 4:6, :], in_=Tt[:, 1:3, :],
                                 func=act.Copy)            # ext_u (x,y)

            # ---- Pool lane: cutoff polynomial (z = (r/rc)^2) -----------
            nc.gpsimd.tensor_scalar(out=zc[:, :], in0=rsq[:, :],
                                    scalar1=(SCL / RC) ** 2, scalar2=1.0,
                                    op0=op.mult, op1=op.min)
            nc.gpsimd.tensor_tensor(out=z2[:, :], in0=zc[:, :], in1=zc[:, :],
                                    op=op.mult)
            nc.vector.scalar_tensor_tensor(out=maskc[:, :], in0=rsq[:, :],
                                           scalar=(RC / SCL) ** 2, in1=mask,
                                           op0=op.is_lt, op1=op.mult)
            nc.vector.tensor_scalar(out=zc21[:, :], in0=zc[:, :],
                                    scalar1=2.0, scalar2=1.0,
                                    op0=op.mult, op1=op.add)
            nc.scalar.activation(out=e0[:, :], in_=zc[:, :], func=act.Copy,
                                 scale=PA1, bias=PA0)
            nc.scalar.activation(out=e1[:, :], in_=zc[:, :], func=act.Copy,
                                 scale=PA3, bias=PA2)
            nc.gpsimd.tensor_scalar(out=f1[:, :], in0=z2[:, :],
                                    scalar1=PA4, scalar2=None, op0=op.mult)
            nc.gpsimd.tensor_tensor(out=f1[:, :], in0=f1[:, :],
                                    in1=e1[:, :], op=op.add)
            nc.gpsimd.tensor_tensor(out=p_[:, :], in0=z2[:, :], in1=f1[:, :],
                                    op=op.mult)
            nc.gpsimd.tensor_tensor(out=cv[:, :], in0=p_[:, :], in1=e0[:, :],
                                    op=op.add)
            nc.gpsimd.tensor_tensor(out=hm[:, :], in0=cv[:, :],
                                    in1=maskc[:, :], op=op.mult)
            nc.gpsimd.tensor_tensor(out=h[:, :], in0=hm[:, :], in1=cv[:, :],
                                    op=op.mult)

            # ---- DVE: x-chain + deg-2 components -----------------------
            nc.vector.tensor_scalar(out=b_[:, :], in0=rij[:, :],
                                    scalar1=SCL, scalar2=RC,
                                    op0=op.mult, op1=op.min)
            nc.vector.tensor_tensor(out=Tt[:, 6:9, :], in0=Tt[:, 1:4, :],
                                    in1=Tt[:, 1:4, :], op=op.mult)  # D
            nc.scalar.activation(out=Tt[:, 9:11, :], in_=Tt[:, 6:8, :],
                                 func=act.Copy)            # ext_D (xx,yy)
            # x = 2*t2 - 1 = 2*zc - 4*min(r,rc)/rc + 1  (zc21 from Pool)
            nc.vector.scalar_tensor_tensor(
                out=Tla[:, 0, :], in0=b_[:, :], scalar=-4.0 / RC,
                in1=zc21[:, :], op0=op.mult, op1=op.add)            # x
            nc.vector.tensor_tensor(out=Tt[:, 11:14, :], in0=Tt[:, 1:4, :],
                                    in1=Tt[:, 2:5, :], op=op.mult)  # R0
            nc.scalar.activation(out=Tt[:, 14:16, :], in_=Tt[:, 11:13, :],
                                 func=act.Copy)            # ext_R (xy,yz)
            nc.vector.tensor_tensor(out=prods[:, 0, :], in0=Tla[:, 0, :],
                                    in1=Tla[:, 0, :], op=op.mult)   # x^2
            nc.vector.tensor_scalar(out=Tla[:, 1, :], in0=prods[:, 0, :],
                                    scalar1=2.0, scalar2=-1.0,
                                    op0=op.mult, op1=op.add)        # T2
            T2b = rowap(Tla, 1, [(0, 2)])
            nc.vector.tensor_tensor(out=prods[:, 0:2, :], in0=Tla[:, 0:2, :],
                                    in1=T2b, op=op.mult)   # xT2, T2^2
            xo = rowap(Tla, 0, [(8, 2)])                   # rows x, ones
            nc.vector.scalar_tensor_tensor(
                out=Tla[:, 2:4, :], in0=prods[:, 0:2, :], scalar=2.0,
                in1=xo, op0=op.mult, op1=op.subtract)      # T3, T4
            nc.vector.tensor_tensor(out=prods[:, 0:2, :], in0=Tla[:, 1:3, :],
                                    in1=Tla[:, 2:4, :],
                                    op=op.mult)            # T2T3, T3T4
            nc.vector.tensor_tensor(out=prods[:, 2:4, :], in0=Tla[:, 2:4, :],
                                    in1=Tla[:, 2:4, :],
                                    op=op.mult)            # T3^2, T4^2
            xb2 = rowap(Tla, 0, [(0, 2)])
            nc.vector.scalar_tensor_tensor(
                out=rowap(Tla, 4, [(2, 2)]), in0=prods[:, 0:2, :],
                scalar=2.0, in1=xb2, op0=op.mult,
                op1=op.subtract)                           # T5, T7
            nc.vector.tensor_scalar(out=rowap(Tla, 5, [(2, 2)]),
                                    in0=prods[:, 2:4, :], scalar1=2.0,
                                    scalar2=-1.0, op0=op.mult,
                                    op1=op.add)            # T6, T8

            # ---- DVE: fused tensor-power groups ------------------------
            Db3 = rowap(Tt, 6, [(0, 3), (1, 3)])
            Db2 = rowap(Tt, 6, [(0, 2), (1, 3)])
            nc.vector.tensor_tensor(out=Tt[:, 16:25, :], in0=Db3,
                                    in1=rowap(Tt, 1, [(1, 3), (1, 3)]),
                                    op=op.mult)            # x3.. yz2
            nc.vector.tensor_tensor(out=Tt[:, 25:31, :], in0=Db2,
                                    in1=rowap(Tt, 6, [(1, 2), (1, 3)]),
                                    op=op.mult)            # x4.. x2z2
            nc.vector.tensor_tensor(out=mov[:, 0, :], in0=h[:, :],
                                    in1=h[:, :], op=op.add)   # phi_0 = 2h
            hb4 = bass.AP(tensor=h[:, :].tensor, offset=h[:, :].offset,
                          ap=[h[:, :].ap[0], [0, 4], [1, W]])
            nc.vector.scalar_tensor_tensor(
                out=mov[:, 1:5, :], in0=Tla[:, 0:4, :], scalar=1.0,
                in1=hb4, op0=op.add, op1=op.mult)          # phi 1..4
            nc.gpsimd.tensor_tensor(out=Tt[:, 31:40, :], in0=Db3,
                                    in1=rowap(Tt, 11, [(1, 3), (1, 3)]),
                                    op=op.mult)            # x3y.. yz3
            nc.vector.tensor_tensor(out=Tt[:, 40, :], in0=Tt[:, 11, :],
                                    in1=Tt[:, 3, :], op=op.mult)  # xyz
            nc.vector.scalar_tensor_tensor(
                out=mov[:, 5:NFEAT, :], in0=Tla[:, 4:8, :], scalar=1.0,
                in1=hb4, op0=op.add, op1=op.mult)          # phi 5..8


            # ---- PE: per-atom moment matmuls + Legendre fold -----------
            for i in range(NI):
                for c in range(NCHUNK):
                    col = c * NI + i
                    nc.tensor.matmul(pm[:, i, :], Tt[:, :, col:col + 1],
                                     mov[:, :, col:col + 1],
                                     start=(c == 0), stop=(c == NCHUNK - 1))
            for i in range(NI):
                for c in range(NCHUNK):
                    col = c * NI + i
                    nc.tensor.matmul(qrT[:, i:i + 1], mov[:, :, col:col + 1],
                                     ones1[:, :],
                                     start=(c == 0), stop=(c == NCHUNK - 1))

            m2v = bass.AP(tensor=OT[:, :].tensor, offset=OT[:, :].offset,
                          ap=[OT[:, :].ap[0], [NA, NI], [1, NA]])
            nc.scalar.activation(out=m2v, in_=pm[:, :, 0:NA],
                                 func=act.Square)
            nc.vector.tensor_copy(out=OT[0:NFEAT, NI * NA:], in_=qrT[:, :])
            nc.sync.dma_start(out=outd.ap()[:, :], in_=OT[:, :])

    nc.compile()
    return nc


def _host_prep(R, box):
    R = np.asarray(R, np.float32)
    box = np.asarray(box, np.float32)
    box_inv = np.linalg.inv(box)
    s = (R @ box_inv.T).astype(np.float64)
    s -= np.floor(s)                                  # fractional in [0,1)
    si_v = s.astype(np.float32)                           # [N,3] fractional
    sj_v = s.astype(np.float32)
    in_maps = []
    for r in range(NCORES):
        ins = np.zeros((128, NCOL), np.float32)
        sl = si_v[r * NI:(r + 1) * NI, :]             # [NI,3]
        for d in range(3):
            ins[:, C_SI + d * NI:C_SI + (d + 1) * NI] = sl[:, d]
        for c in range(NCHUNK):
            for d in range(3):
                ins[:, C_SJ + d * 2 + c] = sj_v[c * 128:(c + 1) * 128, d]
        m = np.full((128, W), 0.5, np.float32)        # 0.5*mask (h scale)
        for i in range(NI):
            g = r * NI + i
            c, j = divmod(g, 128)
            m[j, c * NI + i] = 0.0
        ins[:, C_MASK:C_MASK + W] = m
        in_maps.append({"ins": ins})
    return in_maps


def kernel(R, box):
    R = np.asarray(R)
    box = np.asarray(box)
    key = np.asarray(box, np.float32).tobytes()
    nc = _compiled.get(key)
    if nc is None:
        nc = _build_program(box)
        _compiled[key] = nc
    in_maps = _host_prep(R, box)
    from concourse.bass_utils import run_bass_kernel_spmd
    res = run_bass_kernel_spmd(nc, in_maps, core_ids=list(range(NCORES)))
    A = np.zeros((NCOMP, 5), np.float64)
    for c, (dg, w) in enumerate(_ROWS):
        if dg >= 0:
            A[c] = CLP[:, dg] * w
    parts = []
    for r in range(NCORES):
        ot = res.results[r]["outt"].astype(np.float64)   # [NCOMP, 160]
        qr = ot[0:NFEAT, NI * NA:].T                     # [NI, 9]
        qa = (ot[:, 0:NI * NA].T @ A).reshape(NI, NA * 5)
        parts.append(np.concatenate([qr, qa], axis=1))
    return np.concatenate(parts, axis=0).astype(np.float32)


# revision 44
# speedup vs baseline: 1.0083x; 1.0083x over previous
"""Trainium2 Bass kernel for the atomic-descriptor builder (radial Chebyshev +
angular Legendre descriptors, N=256 atoms, minimum-image PBC).

Strategy: shard the central-atom axis i across 8 NeuronCores (32 atoms each).
Per core, pairs live as [128 j-partitions, 2 j-chunks x 32 atoms free].
The O(N^3) triplet sum is reformulated exactly via the monomial expansion of
Legendre polynomials: q_ang[i,n,l] = sum_c A[c,l] * M[i,n,c]^2 with
M[i,n,c] = sum_j g[i,j,n] (u_ij)^c over the 35 tensor-power monomials of
degree <= 4 (multinomial weights folded into A on the host).

Key layout/scheduling choices (sim-validated against the TRN2 cost model):
  * one packed DVE op for ds (si replicated / sj per-partition views) and a
    two-op fused-compare minimum-image wrap in fractional coordinates; the
    box scale L is absorbed into downstream scalars (sqrt bias, cutoff and
    Chebyshev-argument constants), so no per-element scaling op exists
  * ACT Sqrt is the only table-based activation (table load hidden in the
    input-DMA shadow behind a dependency-free dummy op); the cosine cutoff
    is a degree-4 polynomial in z=(r/rc)^2 (cos(pi*sqrt(z)/2) is entire in
    z), evaluated on the otherwise idle Pool(GPSIMD)+ACT lanes
  * everything after the f32 distance head runs in fp16: TensorTensor and
    TensorScalar get the 2x DVE mode and PE matmuls cost 8ns each
  * the Chebyshev ladder uses product-doubling with row-batched ops
    (multi-row APs computing (T3,T4), (T5,T7), (T6,T8) pairs at once); the
    Chebyshev argument x = 2zc - 4*min(r,rc)/rc + 1 reuses the cutoff's zc
  * the 8 tensor-power "trios" collapse into 3 wide TensorTensor ops using
    sliding-window access patterns over cyclically-extended component rows
    (41-row stationary with 6 duplicate rows, zero-weighted in A); the
    extension copies run on ACT
  * one PSUM bank [41,32,9] accumulates all 64 moment matmuls (start/stop
    per atom); 64 extra 1-column matmuls against a ones vector produce q_r
    partition-transposed as [9,32] so no single-partition copy is needed
  * device tail is one ACT Square (squared moments), one DVE copy, and a
    single merged-output DMA [41,160] fp16; the tiny [35x5] Legendre fold
    A is applied on the host during the unshard/gather (float64)
"""
import numpy as np

N_ATOMS = 256
NCORES = 8
NI = N_ATOMS // NCORES        # 32 central atoms per core
NCHUNK = 2                    # j-chunks of 128 partitions
W = NCHUNK * NI               # 64 free columns per (chunk, atom)
NFEAT = 9                     # radial features (K_RADIAL+1)
NA = 4                        # angular radial features
RC = 5.0
NCOMP = 41                    # 35 unique monomials + 6 cyclic-dup rows
GRP = 8

# fused f32 input block columns: si (once, broadcast on-device) | sj | mask
C_SI, C_SJ, C_MASK = 0, 96, 102
NCOL = C_MASK + W

# cos(pi*y/2) = sum_k PC[k] * (y^2)^k  (Taylor in z=y^2; entire function,
# |err| < 2.6e-5 on z in [0,1])
_PC = [1.0]
for _k in range(1, 5):
    _PC.append(_PC[-1] * (-(np.pi / 2) ** 2) / ((2 * _k - 1) * (2 * _k)))
PA0, PA1, PA2, PA3, PA4 = [float(v) for v in _PC]

# Legendre-in-monomial coefficients: q_l = sum_p CLP[l][p] * S_p
CLP = np.array([
    [1.0, 0, 0, 0, 0],
    [0, 1.0, 0, 0, 0],
    [-0.5, 0, 1.5, 0, 0],
    [0, -1.5, 0, 2.5, 0],
    [0.375, 0, -3.75, 0, 4.375],
], dtype=np.float64)

# stationary component rows: (degree, multinomial weight); -1 deg = dup row
_ROWS = [(0, 1)] + [(1, 1)] * 3 + [(-1, 0)] * 2 \
    + [(2, 1)] * 3 + [(-1, 0)] * 2 + [(2, 2)] * 3 + [(-1, 0)] * 2 \
    + [(3, 1)] * 3 + [(3, 3)] * 6 \
    + [(4, 1)] * 3 + [(4, 6)] * 3 + [(4, 4)] * 3 + [(4, 12)] * 3 \
    + [(4, 4)] * 3 + [(3, 6)]
assert len(_ROWS) == NCOMP


_compiled = {}


def _build_program(box):
    import concourse.bass as bass
    import concourse.bacc as bacc
    import concourse.tile as tile
    from concourse import mybir

    f32 = mybir.dt.float32
    f16 = mybir.dt.float16
    op = mybir.AluOpType
    act = mybir.ActivationFunctionType

    boxf = np.asarray(box, np.float32)
    diag_box = float(np.abs(boxf - np.diag(np.diag(boxf))).max()) == 0.0
    eq_diag = diag_box and boxf[0, 0] == boxf[1, 1] == boxf[2, 2]
    L = float(boxf[0, 0])

    SCL = L if eq_diag else 1.0   # dsw/rsq stay fractional for eq-diag
    nc = bacc.Bacc("TRN2", target_bir_lowering=False, debug=False,
                   enable_asserts=False)

    insd = nc.dram_tensor("ins", [128, NCOL], f32, kind="ExternalInput")
    outd = nc.dram_tensor("outt", [NCOMP, NI * NA + NI], f16,
                      kind="ExternalOutput")

    def rowap(t, r0, pattern, inner=64, cols=slice(0, W)):
        """AP over tile t starting at row r0 with extra row-structured dims.
        pattern = list of (row_step, count); innermost dim = [1, inner]."""
        base = t[:, r0, cols] if inner != W or cols != slice(0, W) \
            else t[:, r0, :]
        rs = t[:, 1, :].offset - t[:, 0, :].offset
        dims = [base.ap[0]] + [[st * rs, n] for st, n in pattern] \
            + [list(base.ap[-1])]
        return bass.AP(tensor=base.tensor, offset=base.offset, ap=dims)

    with tile.TileContext(nc) as tc:
        with tc.tile_pool(name="sb", bufs=1) as sb, \
             tc.tile_pool(name="ps", bufs=1, space="PSUM") as ps:

            def t(shape, tag, dt=f32):
                return sb.tile(shape, dt, tag=tag, name=tag)

            ins = t([128, NCOL], "ins")
            in_ap = insd.ap()
            nc.sync.dma_start(out=ins[:, 0:128], in_=in_ap[:, 0:128])
            nc.sync.dma_start(out=ins[:, 128:], in_=in_ap[:, 128:])
            mask = ins[:, C_MASK:C_MASK + W]

            dsw = t([128, 3, W], "dsw")
            dr2 = t([128, 3, W], "dr2")
            rsq = t([128, W], "rsq")
            rinv = t([128, W], "rinv")
            rij = t([128, W], "rij")
            b_ = t([128, W], "b_")
            zc21 = t([128, W], "zc21")
            Tla = t([128, 9, W], "Tla", f16)   # x T2..T8 | ones
            prods = t([128, 4, W], "prods", f16)
            Tt = t([128, NCOMP, W], "Tt", f16)
            mov = t([128, NFEAT, W], "mov", f16)
            zc = t([128, W], "zc")
            z2 = t([128, W], "z2")
            e0 = t([128, W], "e0")
            e1 = t([128, W], "e1")
            f1 = t([128, W], "f1")
            p_ = t([128, W], "p_")
            cv = t([128, W], "cv")
            maskc = t([128, W], "maskc")
            hm = t([128, W], "hm")
            h = t([128, W], "h", f16)
            OT = t([NCOMP, NI * NA + NI], "OT", f16)
            ones1 = t([128, 1], "ones1", f16)
            b_eps = t([128, 1], "b_eps")

            pm = ps.tile([NCOMP, NI, NFEAT], mybir.dt.float32, tag="pm",
                         name="pm")
            qrT = ps.tile([NFEAT, NI], mybir.dt.float32, tag="qrT",
                          name="qrT")

            # ---- constants (Pool memsets; run in the input-DMA shadow) ----
            nc.gpsimd.memset(Tla[:, 8, :], 1.0)
            nc.gpsimd.memset(Tt[:, 0, :], 1.0)
            nc.gpsimd.memset(b_eps, 1e-12 / SCL ** 2)
            nc.gpsimd.memset(ones1, 1.0)
            nc.gpsimd.memset(OT, 0.0)
            # dep-free first ACT op: forces the single act-table load to run
            # inside the input-DMA shadow instead of behind the rsq wait
            nc.scalar.activation(out=f1[:, 0:1], in_=b_eps[:, :],
                                 func=act.Sqrt, bias=b_eps[:, :])

            # ---- distances (f32 head, DVE) ---------------------------
            # ds = si'' - sj'' in prescaled coords L*(s+1/2) / L*s;
            # minimum image via one python_mod tensor_scalar.
            si_v = bass.AP(tensor=ins[:, :].tensor,
                           offset=ins[:, C_SI:C_SI + 1].offset,
                           ap=[ins[:, :].ap[0], [NI, 3], [0, 2], [1, NI]])
            sj_v = bass.AP(tensor=ins[:, :].tensor,
                           offset=ins[:, C_SJ:C_SJ + 1].offset,
                           ap=[ins[:, :].ap[0], [2, 3], [1, 2], [0, NI]])
            ds4 = bass.AP(tensor=dsw[:, :, :].tensor,
                          offset=dsw[:, :, :].offset,
                          ap=[dsw[:, :, :].ap[0], [W, 3], [NI, 2], [1, NI]])
            nc.vector.tensor_tensor(out=ds4, in0=si_v, in1=sj_v,
                                    op=op.subtract)
            # minimum-image wrap in fractional coords: two fused compare ops
            # (ds>=.5)-ds then (ds<=-.5)-X  == ds - round(ds)
            wrX = dr2                       # reuse dr2 as scratch
            nc.vector.scalar_tensor_tensor(
                out=wrX[:, :, :], in0=dsw[:, :, :], scalar=0.5,
                in1=dsw[:, :, :], op0=op.is_ge, op1=op.subtract)
            nc.vector.scalar_tensor_tensor(
                out=dsw[:, :, :], in0=dsw[:, :, :], scalar=-0.5,
                in1=wrX[:, :, :], op0=op.is_le, op1=op.subtract)
            if not diag_box:
                # general box: dr = B @ ds (fractional wrap already done)
                drt = t([128, 3, W], "drt")
                for d in range(3):
                    nc.vector.tensor_scalar(
                        out=drt[:, d, :], in0=dsw[:, 0, :],
                        scalar1=float(boxf[d, 0]), scalar2=None, op0=op.mult)
                    for e in (1, 2):
                        nc.vector.scalar_tensor_tensor(
                            out=drt[:, d, :], in0=dsw[:, e, :],
                            scalar=float(boxf[d, e]), in1=drt[:, d, :],
                            op0=op.mult, op1=op.add)
                dsw = drt
            elif not eq_diag:
                for d in range(3):
                    nc.vector.tensor_scalar(
                        out=dsw[:, d, :], in0=dsw[:, d, :],
                        scalar1=float(boxf[d, d]), scalar2=None, op0=op.mult)
            nc.vector.tensor_tensor(out=dr2[:, :, :], in0=dsw[:, :, :],
                                    in1=dsw[:, :, :], op=op.mult)
            nc.vector.tensor_reduce(
                out=rsq[:, :], in_=dr2[:, :, :].rearrange("p d w -> p w d"),
                axis=mybir.AxisListType.X, op=op.add)

            # ---- juncture: sqrt (ACT) + reciprocal + unit vectors ------
            nc.scalar.activation(out=rij[:, :], in_=rsq[:, :],
                                 func=act.Sqrt, bias=b_eps[:, :])
            nc.vector.reciprocal(out=rinv[:, :], in_=rij[:, :])
            rinv_b = bass.AP(tensor=rinv[:, :].tensor,
                             offset=rinv[:, :].offset,
                             ap=[rinv[:, :].ap[0], [0, 3], [1, W]])
            nc.vector.tensor_tensor(out=Tt[:, 1:4, :], in0=dsw[:, :, :],
                                    in1=rinv_b, op=op.mult)        # u
            nc.scalar.activation(out=Tt[:, 4:6, :], in_=Tt[:, 1:3, :],
                                 func=act.Copy)            # ext_u (x,y)

            # ---- Pool lane: cutoff polynomial (z = (r/rc)^2) -----------
            nc.gpsimd.tensor_scalar(out=zc[:, :], in0=rsq[:, :],
                                    scalar1=(SCL / RC) ** 2, scalar2=1.0,
                                    op0=op.mult, op1=op.min)
            nc.gpsimd.tensor_tensor(out=z2[:, :], in0=zc[:, :], in1=zc[:, :],
                                    op=op.mult)
            nc.vector.scalar_tensor_tensor(out=maskc[:, :], in0=rsq[:, :],
                                           scalar=(RC / SCL) ** 2, in1=mask,
                                           op0=op.is_lt, op1=op.mult)
            nc.vector.tensor_scalar(out=zc21[:, :], in0=zc[:, :],
                                    scalar1=2.0, scalar2=1.0,
                                    op0=op.mult, op1=op.add)
            nc.scalar.activation(out=e0[:, :], in_=zc[:, :], func=act.Copy,
                                 scale=PA1, bias=PA0)
            nc.scalar.activation(out=e1[:, :], in_=zc[:, :], func=act.Copy,
                                 scale=PA3, bias=PA2)
            nc.gpsimd.tensor_scalar(out=f1[:, :], in0=z2[:, :],
                                    scalar1=PA4, scalar2=None, op0=op.mult)
            nc.gpsimd.tensor_tensor(out=f1[:, :], in0=f1[:, :],
                                    in1=e1[:, :], op=op.add)
            nc.gpsimd.tensor_tensor(out=p_[:, :], in0=z2[:, :], in1=f1[:, :],
                                    op=op.mult)
            nc.gpsimd.tensor_tensor(out=cv[:, :], in0=p_[:, :], in1=e0[:, :],
                                    op=op.add)
            nc.gpsimd.tensor_tensor(out=hm[:, :], in0=cv[:, :],
                                    in1=maskc[:, :], op=op.mult)
            nc.gpsimd.tensor_tensor(out=h[:, :], in0=hm[:, :], in1=cv[:, :],
                                    op=op.mult)

            # ---- DVE: x-chain + deg-2 components -----------------------
            nc.vector.tensor_scalar(out=b_[:, :], in0=rij[:, :],
                                    scalar1=SCL, scalar2=RC,
                                    op0=op.mult, op1=op.min)
            nc.vector.tensor_tensor(out=Tt[:, 6:9, :], in0=Tt[:, 1:4, :],
                                    in1=Tt[:, 1:4, :], op=op.mult)  # D
            nc.scalar.activation(out=Tt[:, 9:11, :], in_=Tt[:, 6:8, :],
                                 func=act.Copy)            # ext_D (xx,yy)
            # x = 2*t2 - 1 = 2*zc - 4*min(r,rc)/rc + 1  (zc21 from Pool)
            nc.vector.scalar_tensor_tensor(
                out=Tla[:, 0, :], in0=b_[:, :], scalar=-4.0 / RC,
                in1=zc21[:, :], op0=op.mult, op1=op.add)            # x
            nc.vector.tensor_tensor(out=Tt[:, 11:14, :], in0=Tt[:, 1:4, :],
                                    in1=Tt[:, 2:5, :], op=op.mult)  # R0
            nc.scalar.activation(out=Tt[:, 14:16, :], in_=Tt[:, 11:13, :],
                                 func=act.Copy)            # ext_R (xy,yz)
            nc.vector.tensor_tensor(out=prods[:, 0, :], in0=Tla[:, 0, :],
                                    in1=Tla[:, 0, :], op=op.mult)   # x^2
            nc.vector.tensor_scalar(out=Tla[:, 1, :], in0=prods[:, 0, :],
                                    scalar1=2.0, scalar2=-1.0,
                                    op0=op.mult, op1=op.add)        # T2
            T2b = rowap(Tla, 1, [(0, 2)])
            nc.vector.tensor_tensor(out=prods[:, 0:2, :], in0=Tla[:, 0:2, :],
                                    in1=T2b, op=op.mult)   # xT2, T2^2
            xo = rowap(Tla, 0, [(8, 2)])                   # rows x, ones
            nc.vector.scalar_tensor_tensor(
                out=Tla[:, 2:4, :], in0=prods[:, 0:2, :], scalar=2.0,
                in1=xo, op0=op.mult, op1=op.subtract)      # T3, T4
            nc.vector.tensor_tensor(out=prods[:, 0:2, :], in0=Tla[:, 1:3, :],
                                    in1=Tla[:, 2:4, :],
                                    op=op.mult)            # T2T3, T3T4
            nc.vector.tensor_tensor(out=prods[:, 2:4, :], in0=Tla[:, 2:4, :],
                                    in1=Tla[:, 2:4, :],
                                    op=op.mult)            # T3^2, T4^2
            xb2 = rowap(Tla, 0, [(0, 2)])
            nc.vector.scalar_tensor_tensor(
                out=rowap(Tla, 4, [(2, 2)]), in0=prods[:, 0:2, :],
                scalar=2.0, in1=xb2, op0=op.mult,
                op1=op.subtract)                           # T5, T7
            nc.vector.tensor_scalar(out=rowap(Tla, 5, [(2, 2)]),
                                    in0=prods[:, 2:4, :], scalar1=2.0,
                                    scalar2=-1.0, op0=op.mult,
                                    op1=op.add)            # T6, T8

            # ---- DVE: fused tensor-power groups ------------------------
            Db3 = rowap(Tt, 6, [(0, 3), (1, 3)])
            Db2 = rowap(Tt, 6, [(0, 2), (1, 3)])
            nc.vector.tensor_tensor(out=Tt[:, 16:25, :], in0=Db3,
                                    in1=rowap(Tt, 1, [(1, 3), (1, 3)]),
                                    op=op.mult)            # x3.. yz2
            nc.vector.tensor_tensor(out=Tt[:, 25:31, :], in0=Db2,
                                    in1=rowap(Tt, 6, [(1, 2), (1, 3)]),
                                    op=op.mult)            # x4.. x2z2
            nc.vector.tensor_tensor(out=mov[:, 0, :], in0=h[:, :],
                                    in1=h[:, :], op=op.add)   # phi_0 = 2h
            hb4 = bass.AP(tensor=h[:, :].tensor, offset=h[:, :].offset,
                          ap=[h[:, :].ap[0], [0, 4], [1, W]])
            nc.vector.scalar_tensor_tensor(
                out=mov[:, 1:5, :], in0=Tla[:, 0:4, :], scalar=1.0,
                in1=hb4, op0=op.add, op1=op.mult)          # phi 1..4
            nc.gpsimd.tensor_tensor(out=Tt[:, 31:40, :], in0=Db3,
                                    in1=rowap(Tt, 11, [(1, 3), (1, 3)]),
                                    op=op.mult)            # x3y.. yz3
            nc.vector.tensor_tensor(out=Tt[:, 40, :], in0=Tt[:, 11, :],
                                    in1=Tt[:, 3, :], op=op.mult)  # xyz
            nc.vector.scalar_tensor_tensor(
                out=mov[:, 5:NFEAT, :], in0=Tla[:, 4:8, :], scalar=1.0,
                in1=hb4, op0=op.add, op1=op.mult)          # phi 5..8


            # ---- PE: per-atom moment matmuls + Legendre fold -----------
            for i in range(NI):
                for c in range(NCHUNK):
                    col = c * NI + i
                    nc.tensor.matmul(pm[:, i, :], Tt[:, :, col:col + 1],
                                     mov[:, :, col:col + 1],
                                     start=(c == 0), stop=(c == NCHUNK - 1))
            for i in range(NI):
                for c in range(NCHUNK):
                    col = c * NI + i
                    nc.tensor.matmul(qrT[:, i:i + 1], mov[:, :, col:col + 1],
                                     ones1[:, :],
                                     start=(c == 0), stop=(c == NCHUNK - 1))

            m2v = bass.AP(tensor=OT[:, :].tensor, offset=OT[:, :].offset,
                          ap=[OT[:, :].ap[0], [NA, NI], [1, NA]])
            nc.scalar.activation(out=m2v, in_=pm[:, :, 0:NA],
                                 func=act.Square)
            nc.vector.tensor_copy(out=OT[0:NFEAT, NI * NA:], in_=qrT[:, :])
            nc.sync.dma_start(out=outd.ap()[:, :], in_=OT[:, :])

    nc.compile()
    return nc


def _host_prep(R, box):
    R = np.asarray(R, np.float32)
    box = np.asarray(box, np.float32)
    box_inv = np.linalg.inv(box)
    s = (R @ box_inv.T).astype(np.float64)
    s -= np.floor(s)                                  # fractional in [0,1)
    si_v = s.astype(np.float32)                           # [N,3] fractional
    sj_v = s.astype(np.float32)
    in_maps = []
    for r in range(NCORES):
        ins = np.zeros((128, NCOL), np.float32)
        sl = si_v[r * NI:(r + 1) * NI, :]             # [NI,3]
        for d in range(3):
            ins[:, C_SI + d * NI:C_SI + (d + 1) * NI] = sl[:, d]
        for c in range(NCHUNK):
            for d in range(3):
                ins[:, C_SJ + d * 2 + c] = sj_v[c * 128:(c + 1) * 128, d]
        m = np.full((128, W), 0.5, np.float32)        # 0.5*mask (h scale)
        for i in range(NI):
            g = r * NI + i
            c, j = divmod(g, 128)
            m[j, c * NI + i] = 0.0
        ins[:, C_MASK:C_MASK + W] = m
        in_maps.append({"ins": ins})
    return in_maps


def kernel(R, box):
    R = np.asarray(R)
    box = np.asarray(box)
    key = np.asarray(box, np.float32).tobytes()
    nc = _compiled.get(key)
    if nc is None:
        nc = _build_program(box)
        _compiled[key] = nc
    in_maps = _host_prep(R, box)
    from concourse.bass_utils import run_bass_kernel_spmd
    res = run_bass_kernel_spmd(nc, in_maps, core_ids=list(range(NCORES)))
    A = np.zeros((NCOMP, 5), np.float64)
    for c, (dg, w) in enumerate(_ROWS):
        if dg >= 0:
            A[c] = CLP[:, dg] * w
    parts = []
    for r in range(NCORES):
        ot = res.results[r]["outt"].astype(np.float64)   # [NCOMP, 160]
        qr = ot[0:NFEAT, NI * NA:].T                     # [NI, 9]
        qa = (ot[:, 0:NI * NA].T @ A).reshape(NI, NA * 5)
        parts.append(np.concatenate([qr, qa], axis=1))
    return np.concatenate(parts, axis=0).astype(np.float32)


# revision 45
# speedup vs baseline: 1.0121x; 1.0038x over previous
"""Trainium2 Bass kernel for the atomic-descriptor builder (radial Chebyshev +
angular Legendre descriptors, N=256 atoms, minimum-image PBC).

Strategy: shard the central-atom axis i across 8 NeuronCores (32 atoms each).
Per core, pairs live as [128 j-partitions, 2 j-chunks x 32 atoms free].
The O(N^3) triplet sum is reformulated exactly via the monomial expansion of
Legendre polynomials: q_ang[i,n,l] = sum_c A[c,l] * M[i,n,c]^2 with
M[i,n,c] = sum_j g[i,j,n] (u_ij)^c over the 35 tensor-power monomials of
degree <= 4 (multinomial weights folded into A on the host).

Key layout/scheduling choices (sim-validated against the TRN2 cost model):
  * one packed DVE op for ds (si replicated / sj per-partition views) and a
    two-op fused-compare minimum-image wrap in fractional coordinates; the
    box scale L is absorbed into downstream scalars (sqrt bias, cutoff and
    Chebyshev-argument constants), so no per-element scaling op exists
  * ACT Sqrt is the only table-based activation (table load hidden in the
    input-DMA shadow behind a dependency-free dummy op); the cosine cutoff
    is a degree-4 polynomial in z=(r/rc)^2 (cos(pi*sqrt(z)/2) is entire in
    z), evaluated on the otherwise idle Pool(GPSIMD)+ACT lanes
  * everything after the f32 distance head runs in fp16: TensorTensor and
    TensorScalar get the 2x DVE mode and PE matmuls cost 8ns each
  * the Chebyshev ladder uses product-doubling with row-batched ops
    (multi-row APs computing (T3,T4), (T5,T7), (T6,T8) pairs at once); the
    Chebyshev argument x = 2zc - 4*min(r,rc)/rc + 1 reuses the cutoff's zc
  * the 8 tensor-power "trios" collapse into 3 wide TensorTensor ops using
    sliding-window access patterns over cyclically-extended component rows
    (41-row stationary with 6 duplicate rows, zero-weighted in A); the
    extension copies run on ACT
  * one PSUM bank [41,32,9] accumulates all 64 moment matmuls (start/stop
    per atom); 64 extra 1-column matmuls against a ones vector produce q_r
    partition-transposed as [9,32] so no single-partition copy is needed
  * device tail is one ACT Square (squared moments), one DVE copy, and a
    single merged-output DMA [41,160] fp16; the tiny [35x5] Legendre fold
    A is applied on the host during the unshard/gather (float64)
"""
import numpy as np

N_ATOMS = 256
NCORES = 8
NI = N_ATOMS // NCORES        # 32 central atoms per core
NCHUNK = 2                    # j-chunks of 128 partitions
W = NCHUNK * NI               # 64 free columns per (chunk, atom)
NFEAT = 9                     # radial features (K_RADIAL+1)
NA = 4                        # angular radial features
RC = 5.0
NCOMP = 41                    # 35 unique monomials + 6 cyclic-dup rows
GRP = 8

# fused f32 input block columns: si (once, broadcast on-device) | sj | mask
C_SI, C_SJ, C_MASK = 0, 96, 102
NCOL = C_MASK + W

# cos(pi*y/2) = sum_k PC[k] * (y^2)^k  (Taylor in z=y^2; entire function,
# |err| < 2.6e-5 on z in [0,1])
_PC = [1.0]
for _k in range(1, 5):
    _PC.append(_PC[-1] * (-(np.pi / 2) ** 2) / ((2 * _k - 1) * (2 * _k)))
PA0, PA1, PA2, PA3, PA4 = [float(v) for v in _PC]

# Legendre-in-monomial coefficients: q_l = sum_p CLP[l][p] * S_p
CLP = np.array([
    [1.0, 0, 0, 0, 0],
    [0, 1.0, 0, 0, 0],
    [-0.5, 0, 1.5, 0, 0],
    [0, -1.5, 0, 2.5, 0],
    [0.375, 0, -3.75, 0, 4.375],
], dtype=np.float64)

# stationary component rows: (degree, multinomial weight); -1 deg = dup row
_ROWS = [(0, 1)] + [(1, 1)] * 3 + [(-1, 0)] * 2 \
    + [(2, 1)] * 3 + [(-1, 0)] * 2 + [(2, 2)] * 3 + [(-1, 0)] * 2 \
    + [(3, 1)] * 3 + [(3, 3)] * 6 \
    + [(4, 1)] * 3 + [(4, 6)] * 3 + [(4, 4)] * 3 + [(4, 12)] * 3 \
    + [(4, 4)] * 3 + [(3, 6)]
assert len(_ROWS) == NCOMP


_compiled = {}


def _build_program(box):
    import concourse.bass as bass
    import concourse.bacc as bacc
    import concourse.tile as tile
    from concourse import mybir

    f32 = mybir.dt.float32
    f16 = mybir.dt.float16
    op = mybir.AluOpType
    act = mybir.ActivationFunctionType

    boxf = np.asarray(box, np.float32)
    diag_box = float(np.abs(boxf - np.diag(np.diag(boxf))).max()) == 0.0
    eq_diag = diag_box and boxf[0, 0] == boxf[1, 1] == boxf[2, 2]
    L = float(boxf[0, 0])

    SCL = L if eq_diag else 1.0   # dsw/rsq stay fractional for eq-diag
    nc = bacc.Bacc("TRN2", target_bir_lowering=False, debug=False,
                   enable_asserts=False)

    insd = nc.dram_tensor("ins", [128, NCOL], f32, kind="ExternalInput")
    outd = nc.dram_tensor("outt", [NCOMP, NI * NA + NI], f16,
                      kind="ExternalOutput")

    def rowap(t, r0, pattern, inner=64, cols=slice(0, W)):
        """AP over tile t starting at row r0 with extra row-structured dims.
        pattern = list of (row_step, count); innermost dim = [1, inner]."""
        base = t[:, r0, cols] if inner != W or cols != slice(0, W) \
            else t[:, r0, :]
        rs = t[:, 1, :].offset - t[:, 0, :].offset
        dims = [base.ap[0]] + [[st * rs, n] for st, n in pattern] \
            + [list(base.ap[-1])]
        return bass.AP(tensor=base.tensor, offset=base.offset, ap=dims)

    with tile.TileContext(nc) as tc:
        with tc.tile_pool(name="sb", bufs=1) as sb, \
             tc.tile_pool(name="ps", bufs=1, space="PSUM") as ps:

            def t(shape, tag, dt=f32):
                return sb.tile(shape, dt, tag=tag, name=tag)

            ins = t([128, NCOL], "ins")
            in_ap = insd.ap()
            nc.sync.dma_start(out=ins[:, 0:128], in_=in_ap[:, 0:128])
            nc.sync.dma_start(out=ins[:, 128:], in_=in_ap[:, 128:])
            mask = ins[:, C_MASK:C_MASK + W]

            dsw = t([128, 3, W], "dsw")
            dr2 = t([128, 3, W], "dr2")
            rsq = t([128, W], "rsq")
            rinv = t([128, W], "rinv")
            rij = t([128, W], "rij")
            b_ = t([128, W], "b_")
            zc21 = t([128, W], "zc21")
            Tla = t([128, 9, W], "Tla", f16)   # x T2..T8 | ones
            prods = t([128, 4, W], "prods", f16)
            Tt = t([128, NCOMP, W], "Tt", f16)
            mov = t([128, NFEAT, W], "mov", f16)
            zc = t([128, W], "zc")
            z2 = t([128, W], "z2")
            e0 = t([128, W], "e0")
            e1 = t([128, W], "e1")
            f1 = t([128, W], "f1")
            p_ = t([128, W], "p_")
            cv = t([128, W], "cv")
            maskc = t([128, W], "maskc")
            hm = t([128, W], "hm")
            h = t([128, W], "h", f16)
            OT = t([NCOMP, NI * NA + NI], "OT", f16)
            ones1 = t([128, 1], "ones1", f16)
            b_eps = t([128, 1], "b_eps")

            pm = ps.tile([NCOMP, NI, NFEAT], mybir.dt.float32, tag="pm",
                         name="pm")
            qrT = ps.tile([NFEAT, NI], mybir.dt.float32, tag="qrT",
                          name="qrT")

            # ---- constants (Pool memsets; run in the input-DMA shadow) ----
            nc.gpsimd.memset(Tla[:, 8, :], 1.0)
            nc.gpsimd.memset(Tt[:, 0, :], 1.0)
            nc.gpsimd.memset(b_eps, 1e-12 / SCL ** 2)
            nc.gpsimd.memset(ones1, 1.0)
            nc.gpsimd.memset(OT, 0.0)
            # dep-free first ACT op: forces the single act-table load to run
            # inside the input-DMA shadow instead of behind the rsq wait
            nc.scalar.activation(out=f1[:, 0:1], in_=b_eps[:, :],
                                 func=act.Sqrt, bias=b_eps[:, :])

            # ---- distances (f32 head, DVE) ---------------------------
            # ds = si'' - sj'' in prescaled coords L*(s+1/2) / L*s;
            # minimum image via one python_mod tensor_scalar.
            si_v = bass.AP(tensor=ins[:, :].tensor,
                           offset=ins[:, C_SI:C_SI + 1].offset,
                           ap=[ins[:, :].ap[0], [NI, 3], [0, 2], [1, NI]])
            sj_v = bass.AP(tensor=ins[:, :].tensor,
                           offset=ins[:, C_SJ:C_SJ + 1].offset,
                           ap=[ins[:, :].ap[0], [2, 3], [1, 2], [0, NI]])
            ds4 = bass.AP(tensor=dsw[:, :, :].tensor,
                          offset=dsw[:, :, :].offset,
                          ap=[dsw[:, :, :].ap[0], [W, 3], [NI, 2], [1, NI]])
            # half-width pipelined spine: each op split into column halves
            # so the engine stays busy through dependent-op ack windows and
            # ACT's sqrt starts after the first rsq half
            wrX = dr2                       # reuse dr2 as scratch
            H0, H1 = slice(0, NI), slice(NI, W)
            for hs in (H0, H1):
                ds4h = bass.AP(tensor=dsw[:, :, hs].tensor,
                               offset=dsw[:, :, hs].offset,
                               ap=[dsw[:, :, hs].ap[0], [W, 3], [1, NI]])
                si_h = bass.AP(tensor=ins[:, :].tensor,
                               offset=ins[:, C_SI:C_SI + 1].offset,
                               ap=[ins[:, :].ap[0], [NI, 3], [1, NI]])
                sj_h = bass.AP(
                    tensor=ins[:, :].tensor,
                    offset=ins[:, C_SJ + (0 if hs == H0 else 1):].offset,
                    ap=[ins[:, :].ap[0], [2, 3], [0, NI]])
                nc.vector.tensor_tensor(out=ds4h, in0=si_h, in1=sj_h,
                                        op=op.subtract)
            for hs in (H0, H1):
                nc.vector.scalar_tensor_tensor(
                    out=wrX[:, :, hs], in0=dsw[:, :, hs], scalar=0.5,
                    in1=dsw[:, :, hs], op0=op.is_ge, op1=op.subtract)
            for hs in (H0, H1):
                nc.vector.scalar_tensor_tensor(
                    out=dsw[:, :, hs], in0=dsw[:, :, hs], scalar=-0.5,
                    in1=wrX[:, :, hs], op0=op.is_le, op1=op.subtract)
            if not diag_box:
                # general box: dr = B @ ds (fractional wrap already done)
                drt = t([128, 3, W], "drt")
                for d in range(3):
                    nc.vector.tensor_scalar(
                        out=drt[:, d, :], in0=dsw[:, 0, :],
                        scalar1=float(boxf[d, 0]), scalar2=None, op0=op.mult)
                    for e in (1, 2):
                        nc.vector.scalar_tensor_tensor(
                            out=drt[:, d, :], in0=dsw[:, e, :],
                            scalar=float(boxf[d, e]), in1=drt[:, d, :],
                            op0=op.mult, op1=op.add)
                dsw = drt
            elif not eq_diag:
                for d in range(3):
                    nc.vector.tensor_scalar(
                        out=dsw[:, d, :], in0=dsw[:, d, :],
                        scalar1=float(boxf[d, d]), scalar2=None, op0=op.mult)
            for hs in (H0, H1):
                nc.vector.tensor_tensor(out=dr2[:, :, hs], in0=dsw[:, :, hs],
                                        in1=dsw[:, :, hs], op=op.mult)
                nc.vector.tensor_reduce(
                    out=rsq[:, hs],
                    in_=dr2[:, :, hs].rearrange("p d w -> p w d"),
                    axis=mybir.AxisListType.X, op=op.add)

            # ---- juncture: sqrt (ACT) + reciprocal + unit vectors ------
            for hs in (H0, H1):
                nc.scalar.activation(out=rij[:, hs], in_=rsq[:, hs],
                                     func=act.Sqrt, bias=b_eps[:, :])
            for hs in (H0, H1):
                nc.vector.reciprocal(out=rinv[:, hs], in_=rij[:, hs])
            rinv_b = bass.AP(tensor=rinv[:, :].tensor,
                             offset=rinv[:, :].offset,
                             ap=[rinv[:, :].ap[0], [0, 3], [1, W]])
            for hs in (H0, H1):
                rb = bass.AP(tensor=rinv[:, hs].tensor,
                             offset=rinv[:, hs].offset,
                             ap=[rinv[:, hs].ap[0], [0, 3], [1, NI]])
                nc.vector.tensor_tensor(out=Tt[:, 1:4, hs],
                                        in0=dsw[:, :, hs],
                                        in1=rb, op=op.mult)        # u
            nc.scalar.activation(out=Tt[:, 4:6, :], in_=Tt[:, 1:3, :],
                                 func=act.Copy)            # ext_u (x,y)

            # ---- Pool lane: cutoff polynomial (z = (r/rc)^2) -----------
            nc.gpsimd.tensor_scalar(out=zc[:, :], in0=rsq[:, :],
                                    scalar1=(SCL / RC) ** 2, scalar2=1.0,
                                    op0=op.mult, op1=op.min)
            nc.gpsimd.tensor_tensor(out=z2[:, :], in0=zc[:, :], in1=zc[:, :],
                                    op=op.mult)
            nc.vector.scalar_tensor_tensor(out=maskc[:, :], in0=rsq[:, :],
                                           scalar=(RC / SCL) ** 2, in1=mask,
                                           op0=op.is_lt, op1=op.mult)
            nc.vector.tensor_scalar(out=zc21[:, :], in0=zc[:, :],
                                    scalar1=2.0, scalar2=1.0,
                                    op0=op.mult, op1=op.add)
            nc.scalar.activation(out=e0[:, :], in_=zc[:, :], func=act.Copy,
                                 scale=PA1, bias=PA0)
            nc.scalar.activation(out=e1[:, :], in_=zc[:, :], func=act.Copy,
                                 scale=PA3, bias=PA2)
            nc.gpsimd.tensor_scalar(out=f1[:, :], in0=z2[:, :],
                                    scalar1=PA4, scalar2=None, op0=op.mult)
            nc.gpsimd.tensor_tensor(out=f1[:, :], in0=f1[:, :],
                                    in1=e1[:, :], op=op.add)
            nc.gpsimd.tensor_tensor(out=p_[:, :], in0=z2[:, :], in1=f1[:, :],
                                    op=op.mult)
            nc.gpsimd.tensor_tensor(out=cv[:, :], in0=p_[:, :], in1=e0[:, :],
                                    op=op.add)
            nc.gpsimd.tensor_tensor(out=hm[:, :], in0=cv[:, :],
                                    in1=maskc[:, :], op=op.mult)
            nc.gpsimd.tensor_tensor(out=h[:, :], in0=hm[:, :], in1=cv[:, :],
                                    op=op.mult)

            # ---- DVE: x-chain + deg-2 components -----------------------
            nc.vector.tensor_scalar(out=b_[:, :], in0=rij[:, :],
                                    scalar1=SCL, scalar2=RC,
                                    op0=op.mult, op1=op.min)
            nc.vector.tensor_tensor(out=Tt[:, 6:9, :], in0=Tt[:, 1:4, :],
                                    in1=Tt[:, 1:4, :], op=op.mult)  # D
            nc.scalar.activation(out=Tt[:, 9:11, :], in_=Tt[:, 6:8, :],
                                 func=act.Copy)            # ext_D (xx,yy)
            # x = 2*t2 - 1 = 2*zc - 4*min(r,rc)/rc + 1  (zc21 from Pool)
            nc.vector.scalar_tensor_tensor(
                out=Tla[:, 0, :], in0=b_[:, :], scalar=-4.0 / RC,
                in1=zc21[:, :], op0=op.mult, op1=op.add)            # x
            nc.vector.tensor_tensor(out=Tt[:, 11:14, :], in0=Tt[:, 1:4, :],
                                    in1=Tt[:, 2:5, :], op=op.mult)  # R0
            nc.scalar.activation(out=Tt[:, 14:16, :], in_=Tt[:, 11:13, :],
                                 func=act.Copy)            # ext_R (xy,yz)
            nc.vector.tensor_tensor(out=prods[:, 0, :], in0=Tla[:, 0, :],
                                    in1=Tla[:, 0, :], op=op.mult)   # x^2
            nc.vector.tensor_scalar(out=Tla[:, 1, :], in0=prods[:, 0, :],
                                    scalar1=2.0, scalar2=-1.0,
                                    op0=op.mult, op1=op.add)        # T2
            T2b = rowap(Tla, 1, [(0, 2)])
            nc.vector.tensor_tensor(out=prods[:, 0:2, :], in0=Tla[:, 0:2, :],
                                    in1=T2b, op=op.mult)   # xT2, T2^2
            xo = rowap(Tla, 0, [(8, 2)])                   # rows x, ones
            nc.vector.scalar_tensor_tensor(
                out=Tla[:, 2:4, :], in0=prods[:, 0:2, :], scalar=2.0,
                in1=xo, op0=op.mult, op1=op.subtract)      # T3, T4
            nc.vector.tensor_tensor(out=prods[:, 0:2, :], in0=Tla[:, 1:3, :],
                                    in1=Tla[:, 2:4, :],
                                    op=op.mult)            # T2T3, T3T4
            nc.vector.tensor_tensor(out=prods[:, 2:4, :], in0=Tla[:, 2:4, :],
                                    in1=Tla[:, 2:4, :],
                                    op=op.mult)            # T3^2, T4^2
            xb2 = rowap(Tla, 0, [(0, 2)])
            nc.vector.scalar_tensor_tensor(
                out=rowap(Tla, 4, [(2, 2)]), in0=prods[:, 0:2, :],
                scalar=2.0, in1=xb2, op0=op.mult,
                op1=op.subtract)                           # T5, T7
            nc.vector.tensor_scalar(out=rowap(Tla, 5, [(2, 2)]),
                                    in0=prods[:, 2:4, :], scalar1=2.0,
                                    scalar2=-1.0, op0=op.mult,
                                    op1=op.add)            # T6, T8

            # ---- DVE: fused tensor-power groups ------------------------
            Db3 = rowap(Tt, 6, [(0, 3), (1, 3)])
            Db2 = rowap(Tt, 6, [(0, 2), (1, 3)])
            nc.vector.tensor_tensor(out=Tt[:, 16:25, :], in0=Db3,
                                    in1=rowap(Tt, 1, [(1, 3), (1, 3)]),
                                    op=op.mult)            # x3.. yz2
            nc.vector.tensor_tensor(out=Tt[:, 25:31, :], in0=Db2,
                                    in1=rowap(Tt, 6, [(1, 2), (1, 3)]),
                                    op=op.mult)            # x4.. x2z2
            nc.vector.tensor_tensor(out=mov[:, 0, :], in0=h[:, :],
                                    in1=h[:, :], op=op.add)   # phi_0 = 2h
            hb4 = bass.AP(tensor=h[:, :].tensor, offset=h[:, :].offset,
                          ap=[h[:, :].ap[0], [0, 4], [1, W]])
            nc.vector.scalar_tensor_tensor(
                out=mov[:, 1:5, :], in0=Tla[:, 0:4, :], scalar=1.0,
                in1=hb4, op0=op.add, op1=op.mult)          # phi 1..4
            nc.gpsimd.tensor_tensor(out=Tt[:, 31:40, :], in0=Db3,
                                    in1=rowap(Tt, 11, [(1, 3), (1, 3)]),
                                    op=op.mult)            # x3y.. yz3
            nc.vector.tensor_tensor(out=Tt[:, 40, :], in0=Tt[:, 11, :],
                                    in1=Tt[:, 3, :], op=op.mult)  # xyz
            nc.vector.scalar_tensor_tensor(
                out=mov[:, 5:NFEAT, :], in0=Tla[:, 4:8, :], scalar=1.0,
                in1=hb4, op0=op.add, op1=op.mult)          # phi 5..8


            # ---- PE: per-atom moment matmuls + Legendre fold -----------
            for i in range(NI):
                for c in range(NCHUNK):
                    col = c * NI + i
                    nc.tensor.matmul(pm[:, i, :], Tt[:, :, col:col + 1],
                                     mov[:, :, col:col + 1],
                                     start=(c == 0), stop=(c == NCHUNK - 1))
            for i in range(NI):
                for c in range(NCHUNK):
                    col = c * NI + i
                    nc.tensor.matmul(qrT[:, i:i + 1], mov[:, :, col:col + 1],
                                     ones1[:, :],
                                     start=(c == 0), stop=(c == NCHUNK - 1))

            m2v = bass.AP(tensor=OT[:, :].tensor, offset=OT[:, :].offset,
                          ap=[OT[:, :].ap[0], [NA, NI], [1, NA]])
            nc.scalar.activation(out=m2v, in_=pm[:, :, 0:NA],
                                 func=act.Square)
            nc.vector.tensor_copy(out=OT[0:NFEAT, NI * NA:], in_=qrT[:, :])
            nc.sync.dma_start(out=outd.ap()[:, :], in_=OT[:, :])

    nc.compile()
    return nc


def _host_prep(R, box):
    R = np.asarray(R, np.float32)
    box = np.asarray(box, np.float32)
    box_inv = np.linalg.inv(box)
    s = (R @ box_inv.T).astype(np.float64)
    s -= np.floor(s)                                  # fractional in [0,1)
    si_v = s.astype(np.float32)                           # [N,3] fractional
    sj_v = s.astype(np.float32)
    in_maps = []
    for r in range(NCORES):
        ins = np.zeros((128, NCOL), np.float32)
        sl = si_v[r * NI:(r + 1) * NI, :]             # [NI,3]
        for d in range(3):
            ins[:, C_SI + d * NI:C_SI + (d + 1) * NI] = sl[:, d]
        for c in range(NCHUNK):
            for d in range(3):
                ins[:, C_SJ + d * 2 + c] = sj_v[c * 128:(c + 1) * 128, d]
        m = np.full((128, W), 0.5, np.float32)        # 0.5*mask (h scale)
        for i in range(NI):
            g = r * NI + i
            c, j = divmod(g, 128)
            m[j, c * NI + i] = 0.0
        ins[:, C_MASK:C_MASK + W] = m
        in_maps.append({"ins": ins})
    return in_maps


def kernel(R, box):
    R = np.asarray(R)
    box = np.asarray(box)
    key = np.asarray(box, np.float32).tobytes()
    nc = _compiled.get(key)
    if nc is None:
        nc = _build_program(box)
        _compiled[key] = nc
    in_maps = _host_prep(R, box)
    from concourse.bass_utils import run_bass_kernel_spmd
    res = run_bass_kernel_spmd(nc, in_maps, core_ids=list(range(NCORES)))
    A = np.zeros((NCOMP, 5), np.float64)
    for c, (dg, w) in enumerate(_ROWS):
        if dg >= 0:
            A[c] = CLP[:, dg] * w
    parts = []
    for r in range(NCORES):
        ot = res.results[r]["outt"].astype(np.float64)   # [NCOMP, 160]
        qr = ot[0:NFEAT, NI * NA:].T                     # [NI, 9]
        qa = (ot[:, 0:NI * NA].T @ A).reshape(NI, NA * 5)
        parts.append(np.concatenate([qr, qa], axis=1))
    return np.concatenate(parts, axis=0).astype(np.float32)
